# revision 1
# baseline (speedup 1.0000x reference)
"""Bass/Trainium2 kernel for nn_EF_42511586295882 (GNN message passing).

Math reduction proven against reference: only the l=0 spherical channel of
iteration 0 reaches the output (refinement mixes features, never l-channels,
and only x[:, 0, :] feeds iteration 1 / readout).  The whole computation is:

  rad[e,k]  = T_k(2*exp(-r)-1) * cut(r) * valid_mask          (E,16)
  msg0[e,f] = (rad @ (0.282095*Wr1_0 + Wr2_0))[e,f] * embed[z[src_e], f]
  X0[a,f]   = sum_{e: dst=a} msg0[e,f]
  x0        = X0 + (h0 * silu(h0)) @ W2_0,   h0 = X0 @ W1_0
  msg1[e,f] = (rad @ Wr1_1)[e,f] * x0[src_e, f]
  X1[a,f]   = sum_{e: dst=a} msg1[e,f]
  x0b       = X1 + silu(X1 @ W1_1) @ W2_1
  e_atom    = x0b @ w_out + b_out[z] + sum_{e: dst=a} e_pair[e]
  e_mol     = segment_sum(e_atom * atom_mask, batch_segments)

Sharding: edges sorted by dst; core k owns atoms [2048k, 2048(k+1)) and all
edges into them, grouped in 16 aligned 128-atom blocks.  Scatter = one-hot
matmul into a per-block PSUM accumulator.  x0 is exchanged with an AllGather
between the two message-passing passes.
"""

import math
import numpy as np

P = 128
N = 16384
E = 262144
B = 512
F = 32
K = 16
NZ = 119
NCORES = 8
AC = N // NCORES          # atoms per core
NB = AC // P              # 128-atom blocks per core (16)
CUTOFF = 6.0
KE = 14.399645
ZBL_C = [0.18175, 0.50986, 0.28022, 0.02817]
ZBL_D = [3.19980, 0.94229, 0.40290, 0.20162]
A_PRE = 0.8854 * 0.529177

_CACHE = {}


def _host_prep(atomic_numbers, positions, dst_idx, src_idx, batch_segments,
               batch_mask, atom_mask, embed, Wr1_0, Wr2_0, W1_0, W2_0,
               Wr1_1, W1_1, W2_1, w_out, b_out):
    an = np.asarray(atomic_numbers).astype(np.int32)
    pos = np.asarray(positions, dtype=np.float32)
    dst = np.asarray(dst_idx).astype(np.int64)
    src = np.asarray(src_idx).astype(np.int64)
    seg = np.asarray(batch_segments).astype(np.int64)

    order = np.argsort(dst, kind="stable")
    dsts, srcs = dst[order], src[order]

    core_of = dsts // AC
    blk_of = (dsts % AC) // P

    # per (core, block) edge lists
    counts = np.zeros((NCORES, NB), dtype=np.int64)
    for c in range(NCORES):
        m = core_of == c
        cb = np.bincount(blk_of[m], minlength=NB)
        counts[c] = cb
    T_blk = int(math.ceil(counts.max() / P))
    T = NB * T_blk

    dstloc = np.zeros((NCORES, P, T), dtype=np.float32)
    dsti = np.zeros((NCORES, P, T), dtype=np.int32)
    srci = np.zeros((NCORES, P, T), dtype=np.int32)
    zsrci = np.zeros((NCORES, P, T), dtype=np.int32)

    # fill per core/block; padded slots keep zeros (masked via rad=0: we set
    # their radial seed cutm to 0 by pointing src=dst=0 and forcing r... the
    # mask is folded multiplicatively into the radial seed on device, driven
    # by maskd below)
    maskd = np.zeros((NCORES, P, T), dtype=np.float32)
    edge_pos = np.argsort(core_of * NB + blk_of, kind="stable")
    ptr = 0
    for c in range(NCORES):
        for b in range(NB):
            n = counts[c, b]
            idx = edge_pos[ptr:ptr + n]
            ptr += n
            k = np.arange(n)
            t = b * T_blk + (k // P)
            p = k % P
            dstloc[c, p, t] = (dsts[idx] % P).astype(np.float32)
            dsti[c, p, t] = dsts[idx]
            srci[c, p, t] = srcs[idx]
            zsrci[c, p, t] = an[srcs[idx]]
            maskd[c, p, t] = 1.0

    # packed per-atom table [px,py,pz,zf,zpow,0,0,0] ; zpow from a 119-entry
    # constant LUT (z^0.23)
    zpow_tab = (np.arange(NZ, dtype=np.float32) ** 0.23).astype(np.float32)
    pat = np.zeros((N, 8), dtype=np.float32)
    pat[:, 0:3] = pos
    pat[:, 3] = an.astype(np.float32)
    pat[:, 4] = zpow_tab[an]

    embedp = np.zeros((1024, F), dtype=np.float32)
    embedp[:NZ] = np.asarray(embed, dtype=np.float32)

    gcW = 0.282095 * np.asarray(Wr1_0, np.float32) + np.asarray(Wr2_0, np.float32)
    wcat = np.zeros((P, 2 * F), dtype=np.float32)  # replicated at 32-row groups
    for j in range(4):
        wcat[32 * j:32 * j + K, 0:F] = gcW
        wcat[32 * j:32 * j + K, F:2 * F] = np.asarray(Wr1_1, np.float32)

    iota_rep = np.tile(np.arange(P, dtype=np.float32)[None, :], (P, 1))
    wout_rep = np.tile(np.asarray(w_out, np.float32)[None, :], (P, 1))

    # per-atom (owned) arrays, layout (P, NB): atom a=128*b+p of the core
    a_all = np.arange(N)
    ownz = an[a_all].reshape(NCORES, NB, P).transpose(0, 2, 1).astype(np.int32)
    segv = seg[a_all].reshape(NCORES, NB, P).transpose(0, 2, 1)
    mol_base = segv.min(axis=(1, 2))
    segloc = (segv - mol_base[:, None, None]).astype(np.float32)
    assert segloc.max() < P, "molecule window exceeds 128 per core"
    amask = np.asarray(atom_mask, np.float32).reshape(NCORES, NB, P).transpose(0, 2, 1)

    boutc = np.asarray(b_out, np.float32).reshape(NZ, 1)

    embf = np.asarray(embed, dtype=np.float32)
    pdall = pat[dsti]                       # (NCORES, P, T, 8)
    psall = pat[srci]
    xs0all = embf[np.clip(zsrci, 0, NZ - 1)]  # (NCORES, P, T, F)
    boutg = np.asarray(b_out, np.float32)[ownz]  # (NCORES, P, NB)

    per_core = []
    for c in range(NCORES):
        per_core.append({
            "dstloc": dstloc[c], "maskd": maskd[c],
            "pdall": pdall[c].reshape(P, -1), "psall": psall[c].reshape(P, -1),
            "xs0all": xs0all[c].reshape(P, -1), "wcat": wcat,
            "iota_rep": iota_rep, "wout_rep": wout_rep,
            "w10": np.asarray(W1_0, np.float32), "w20": np.asarray(W2_0, np.float32),
            "w11": np.asarray(W1_1, np.float32), "w21": np.asarray(W2_1, np.float32),
            "boutg": boutg[c], "segloc": segloc[c], "amask": amask[c],
        })
    return per_core, T, T_blk, mol_base, srci


def _build_A(T, T_blk):
    import concourse.bacc as bacc
    import concourse.bass as bass
    import concourse.mybir as mybir
    import concourse.tile as tile
    from concourse.masks import make_identity

    f32 = mybir.dt.float32
    i32 = mybir.dt.int32
    ALU = mybir.AluOpType
    ACT = mybir.ActivationFunctionType

    nc = bacc.Bacc("TRN2", target_bir_lowering=False, debug=False,
                   num_devices=NCORES)

    # ---- I/O ----
    d_dstloc = nc.dram_tensor("dstloc", [P, T], f32, kind="ExternalInput")
    d_maskd = nc.dram_tensor("maskd", [P, T], f32, kind="ExternalInput")
    d_pd = nc.dram_tensor("pdall", [P, T * 8], f32, kind="ExternalInput")
    d_ps = nc.dram_tensor("psall", [P, T * 8], f32, kind="ExternalInput")
    d_xs0 = nc.dram_tensor("xs0all", [P, T * F], f32, kind="ExternalInput")
    d_wcat = nc.dram_tensor("wcat", [P, 2 * F], f32, kind="ExternalInput")
    d_iota = nc.dram_tensor("iota_rep", [P, P], f32, kind="ExternalInput")
    d_woutr = nc.dram_tensor("wout_rep", [P, F], f32, kind="ExternalInput")
    d_w10 = nc.dram_tensor("w10", [F, F], f32, kind="ExternalInput")
    d_w20 = nc.dram_tensor("w20", [F, F], f32, kind="ExternalInput")
    d_w11 = nc.dram_tensor("w11", [F, F], f32, kind="ExternalInput")
    d_w21 = nc.dram_tensor("w21", [F, F], f32, kind="ExternalInput")
    d_x0out = nc.dram_tensor("x0out", [P, NB * F], f32, kind="ExternalOutput")
    d_gout = nc.dram_tensor("gout", [P, T * F], f32, kind="ExternalOutput")
    d_epat = nc.dram_tensor("epat_o", [P, NB], f32, kind="ExternalOutput")

    with tile.TileContext(nc) as tc:
        with tc.tile_pool(name="const", bufs=1) as cpool, \
             tc.tile_pool(name="persist", bufs=1) as pp, \
             tc.tile_pool(name="dram", bufs=1, space="DRAM") as dpool:

            ident = cpool.tile([P, P], f32, tag="ident")
            make_identity(nc, ident[:])
            iota = cpool.tile([P, P], f32, tag="iota")
            nc.sync.dma_start(iota[:], d_iota[:, :])
            wcat = cpool.tile([P, 2 * F], f32, tag="wcat")
            nc.sync.dma_start(wcat[:], d_wcat[:, :])
            woutr = cpool.tile([P, F], f32, tag="woutr")
            nc.sync.dma_start(woutr[:], d_woutr[:, :])
            w10 = cpool.tile([F, F], f32, tag="w10")
            nc.sync.dma_start(w10[:], d_w10[:, :])
            w20 = cpool.tile([F, F], f32, tag="w20")
            nc.sync.dma_start(w20[:], d_w20[:, :])
            w11 = cpool.tile([F, F], f32, tag="w11")
            nc.sync.dma_start(w11[:], d_w11[:, :])
            w21 = cpool.tile([F, F], f32, tag="w21")
            nc.sync.dma_start(w21[:], d_w21[:, :])

            dstloc = pp.tile([P, T], f32, tag="dstloc")
            nc.sync.dma_start(dstloc[:], d_dstloc[:, :])
            maskd = pp.tile([P, T], f32, tag="maskd")
            nc.sync.dma_start(maskd[:], d_maskd[:, :])

            g_all = pp.tile([P, T, F], f32, tag="g_all")
            epair = pp.tile([P, T], f32, tag="epair")
            X0sb = pp.tile([P, NB, F], f32, tag="X0sb")
            epat = pp.tile([P, NB], f32, tag="epat")
            x0sb = pp.tile([P, NB, F], f32, tag="x0sb")

            # ---------------- pass 1: edge batch math ----------------
            with tc.tile_pool(name="p1", bufs=1) as p1, \
                 tc.tile_pool(name="rot", bufs=3) as rot, \
                 tc.tile_pool(name="ps1", bufs=2, space="PSUM") as ps_rt, \
                 tc.tile_pool(name="ps2", bufs=2, space="PSUM") as ps_g, \
                 tc.tile_pool(name="ps3", bufs=2, space="PSUM") as ps_x, \
                 tc.tile_pool(name="ps4", bufs=2, space="PSUM") as ps_e:

                pd = p1.tile([P, T, 8], f32, tag="pd")
                ps_ = p1.tile([P, T, 8], f32, tag="ps")
                xs0 = p1.tile([P, T, F], f32, tag="xs0")
                nc.sync.dma_start(pd[:], d_pd[:, :].rearrange("p (t c) -> p t c", c=8))
                nc.sync.dma_start(ps_[:], d_ps[:, :].rearrange("p (t c) -> p t c", c=8))
                nc.sync.dma_start(xs0[:], d_xs0[:, :].rearrange("p (t c) -> p t c", c=F))

                disp = p1.tile([P, T, 3], f32, tag="disp")
                nc.vector.tensor_tensor(out=disp[:], in0=ps_[:, :, 0:3],
                                        in1=pd[:, :, 0:3], op=ALU.subtract)
                sq = p1.tile([P, T, 3], f32, tag="sq")
                nc.vector.tensor_tensor(out=sq[:], in0=disp[:], in1=disp[:],
                                        op=ALU.mult)
                r2 = p1.tile([P, T], f32, tag="r2")
                nc.vector.tensor_reduce(out=r2[:], in_=sq[:],
                                        axis=mybir.AxisListType.X, op=ALU.add)
                r = p1.tile([P, T], f32, tag="r")
                nc.scalar.activation(out=r[:], in_=r2[:], func=ACT.Sqrt)
                nc.vector.tensor_scalar_max(out=r[:], in0=r[:], scalar1=1e-4)

                # t = 2*exp(-r) - 1
                tch = p1.tile([P, T], f32, tag="tch")
                nc.scalar.activation(out=tch[:], in_=r[:], func=ACT.Exp,
                                     scale=-1.0)
                t2 = p1.tile([P, T], f32, tag="t2")
                nc.vector.tensor_scalar(out=t2[:], in0=tch[:], scalar1=4.0,
                                        scalar2=-2.0, op0=ALU.mult, op1=ALU.add)
                nc.vector.tensor_scalar(out=tch[:], in0=tch[:], scalar1=2.0,
                                        scalar2=-1.0, op0=ALU.mult, op1=ALU.add)

                # cutoff: cut = exp(-u2/(1-u2)), u = min(r/C, 1-1e-6)
                u = p1.tile([P, T], f32, tag="u")
                nc.vector.tensor_scalar(out=u[:], in0=r[:],
                                        scalar1=1.0 / CUTOFF,
                                        scalar2=1.0 - 1e-6,
                                        op0=ALU.mult, op1=ALU.min)
                u2 = p1.tile([P, T], f32, tag="u2")
                nc.vector.tensor_tensor(out=u2[:], in0=u[:], in1=u[:],
                                        op=ALU.mult)
                den = p1.tile([P, T], f32, tag="den")
                nc.vector.tensor_scalar(out=den[:], in0=u2[:], scalar1=-1.0,
                                        scalar2=1.0, op0=ALU.mult, op1=ALU.add)
                nc.vector.reciprocal(out=den[:], in_=den[:])
                frac = p1.tile([P, T], f32, tag="frac")
                nc.vector.tensor_tensor(out=frac[:], in0=u2[:], in1=den[:],
                                        op=ALU.mult)
                cutm = p1.tile([P, T], f32, tag="cutm")
                nc.scalar.activation(out=cutm[:], in_=frac[:], func=ACT.Exp,
                                     scale=-1.0)
                nc.vector.tensor_tensor(out=cutm[:], in0=cutm[:], in1=maskd[:],
                                        op=ALU.mult)

                # Chebyshev ladder, seeded with cutm so rad_k = T_k(t)*cut*mask
                rad = p1.tile([P, T, 2 * K], f32, tag="rad")
                nc.vector.memset(rad[:], 0.0)
                nc.vector.tensor_copy(out=rad[:, :, 0], in_=cutm[:])
                nc.vector.tensor_tensor(out=rad[:, :, 1], in0=tch[:],
                                        in1=cutm[:], op=ALU.mult)
                tmp = p1.tile([P, T], f32, tag="tmp")
                for k in range(2, K):
                    nc.vector.tensor_tensor(out=tmp[:], in0=t2[:],
                                            in1=rad[:, :, k - 1], op=ALU.mult)
                    nc.vector.tensor_tensor(out=rad[:, :, k], in0=tmp[:],
                                            in1=rad[:, :, k - 2],
                                            op=ALU.subtract)

                # ---- ZBL pair energy ----
                zz = p1.tile([P, T], f32, tag="zz")
                nc.vector.tensor_tensor(out=zz[:], in0=pd[:, :, 3],
                                        in1=ps_[:, :, 3], op=ALU.mult)
                asum = p1.tile([P, T], f32, tag="asum")
                nc.vector.tensor_tensor(out=asum[:], in0=pd[:, :, 4],
                                        in1=ps_[:, :, 4], op=ALU.add)
                nc.vector.tensor_scalar_add(out=asum[:], in0=asum[:],
                                            scalar1=1e-10)
                ra = p1.tile([P, T], f32, tag="ra")
                nc.vector.tensor_tensor(out=ra[:], in0=r[:], in1=asum[:],
                                        op=ALU.mult)
                nc.vector.tensor_scalar_mul(out=ra[:], in0=ra[:],
                                            scalar1=1.0 / A_PRE)
                phi = p1.tile([P, T], f32, tag="phi")
                ej = p1.tile([P, T], f32, tag="ej")
                for j in range(4):
                    nc.scalar.activation(out=ej[:], in_=ra[:], func=ACT.Exp,
                                         scale=-ZBL_D[j])
                    if j == 0:
                        nc.vector.tensor_scalar_mul(out=phi[:], in0=ej[:],
                                                    scalar1=ZBL_C[j])
                    else:
                        nc.vector.tensor_scalar(out=ej[:], in0=ej[:],
                                                scalar1=ZBL_C[j], scalar2=None,
                                                op0=ALU.mult)
                        nc.vector.tensor_tensor(out=phi[:], in0=phi[:],
                                                in1=ej[:], op=ALU.add)
                rinv = p1.tile([P, T], f32, tag="rinv")
                nc.vector.reciprocal(out=rinv[:], in_=r[:])
                nc.vector.tensor_tensor(out=epair[:], in0=zz[:], in1=phi[:],
                                        op=ALU.mult)
                nc.vector.tensor_tensor(out=epair[:], in0=epair[:], in1=rinv[:],
                                        op=ALU.mult)
                nc.vector.tensor_tensor(out=epair[:], in0=epair[:], in1=cutm[:],
                                        op=ALU.mult)
                nc.vector.tensor_scalar_mul(out=epair[:], in0=epair[:],
                                            scalar1=0.5 * KE)

                # ---------------- pass 1: per-tile scatter ----------------
                for b in range(NB):
                    x0ps = ps_x.tile([P, F + 1], f32, tag="x0ps")
                    for j in range(T_blk):
                        t = b * T_blk + j
                        g4 = t % 4
                        if g4 == 0:
                            radT = ps_rt.tile([P, P], f32, tag="radT")
                            hi = min(4, T - t)
                            nc.tensor.transpose(
                                out=radT[0:32 * hi, :],
                                in_=rad[:, t:t + hi, :],
                                identity=ident[:])
                            radTs = rot.tile([P, P], f32, tag="radTs")
                            nc.scalar.copy(out=radTs[0:32 * hi, :],
                                           in_=radT[0:32 * hi, :])
                        gps = ps_g.tile([P, 2 * F], f32, tag="gps")
                        nc.tensor.matmul(out=gps[:],
                                         lhsT=radTs[32 * g4:32 * g4 + 32, :],
                                         rhs=wcat[32 * g4:32 * g4 + 32, :],
                                         start=True, stop=True,
                                         tile_position=(32 * g4, 0))
                        oh = rot.tile([P, P], f32, tag="oh")
                        nc.vector.tensor_scalar(out=oh[:], in0=iota[:],
                                                scalar1=dstloc[:, t:t + 1],
                                                scalar2=None, op0=ALU.is_equal)
                        msg = rot.tile([P, F + 1], f32, tag="msg")
                        nc.vector.tensor_tensor(out=msg[:, 0:F], in0=gps[:, 0:F],
                                                in1=xs0[:, t, :], op=ALU.mult)
                        nc.vector.tensor_copy(out=msg[:, F:F + 1],
                                              in_=epair[:, t:t + 1])
                        nc.scalar.copy(out=g_all[:, t, :], in_=gps[:, F:2 * F])
                        nc.tensor.matmul(out=x0ps[:], lhsT=oh[:], rhs=msg[:],
                                         start=(j == 0), stop=(j == T_blk - 1))
                    nc.scalar.copy(out=X0sb[:, b, :], in_=x0ps[:, 0:F])
                    nc.vector.tensor_copy(out=epat[:, b:b + 1],
                                          in_=x0ps[:, F:F + 1])

            # ---------------- refinement 0 ----------------
            with tc.tile_pool(name="rf", bufs=2) as rf, \
                 tc.tile_pool(name="rps1", bufs=2, space="PSUM") as rps1, \
                 tc.tile_pool(name="rps2", bufs=2, space="PSUM") as rps2:
                for b in range(NB):
                    trp = rps1.tile([F, P], f32, tag="trp")
                    nc.tensor.transpose(out=trp[:], in_=X0sb[:, b, :],
                                        identity=ident[:])
                    xT = rf.tile([F, P], f32, tag="xT")
                    nc.scalar.copy(out=xT[:], in_=trp[:])
                    hps = rps2.tile([P, F], f32, tag="hps")
                    nc.tensor.matmul(out=hps[:], lhsT=xT[:], rhs=w10[:],
                                     start=True, stop=True)
                    sw = rf.tile([P, F], f32, tag="sw")
                    nc.scalar.activation(out=sw[:], in_=hps[:], func=ACT.Silu)
                    gate = rf.tile([P, F], f32, tag="gate")
                    nc.vector.tensor_tensor(out=gate[:], in0=hps[:], in1=sw[:],
                                            op=ALU.mult)
                    gtp = rps1.tile([F, P], f32, tag="trp")
                    nc.tensor.transpose(out=gtp[:], in_=gate[:],
                                        identity=ident[:])
                    gT = rf.tile([F, P], f32, tag="gT")
                    nc.scalar.copy(out=gT[:], in_=gtp[:])
                    dps = rps2.tile([P, F], f32, tag="hps")
                    nc.tensor.matmul(out=dps[:], lhsT=gT[:], rhs=w20[:],
                                     start=True, stop=True)
                    nc.vector.tensor_tensor(out=x0sb[:, b, :],
                                            in0=X0sb[:, b, :], in1=dps[:],
                                            op=ALU.add)

                nc.sync.dma_start(d_x0out[:, :], x0sb[:])
                nc.sync.dma_start(d_gout[:, :], g_all[:])
                nc.sync.dma_start(d_epat[:, :], epat[:])
    return nc


def _build_B(T, T_blk):
    import concourse.bacc as bacc
    import concourse.bass as bass
    import concourse.mybir as mybir
    import concourse.tile as tile
    from concourse.masks import make_identity

    f32 = mybir.dt.float32
    i32 = mybir.dt.int32
    ALU = mybir.AluOpType
    ACT = mybir.ActivationFunctionType

    nc = bacc.Bacc("TRN2", target_bir_lowering=False, debug=False,
                   num_devices=NCORES)
    d_dstloc = nc.dram_tensor("dstloc", [P, T], f32, kind="ExternalInput")
    d_gall = nc.dram_tensor("gall", [P, T * F], f32, kind="ExternalInput")
    d_epat = nc.dram_tensor("epat_i", [P, NB], f32, kind="ExternalInput")
    d_x0src = nc.dram_tensor("x0src", [P, T * F], f32, kind="ExternalInput")
    d_iota = nc.dram_tensor("iota_rep", [P, P], f32, kind="ExternalInput")
    d_woutr = nc.dram_tensor("wout_rep", [P, F], f32, kind="ExternalInput")
    d_w11 = nc.dram_tensor("w11", [F, F], f32, kind="ExternalInput")
    d_w21 = nc.dram_tensor("w21", [F, F], f32, kind="ExternalInput")
    d_boutg = nc.dram_tensor("boutg", [P, NB], f32, kind="ExternalInput")
    d_segloc = nc.dram_tensor("segloc", [P, NB], f32, kind="ExternalInput")
    d_amask = nc.dram_tensor("amask", [P, NB], f32, kind="ExternalInput")
    d_out = nc.dram_tensor("out", [P, 1], f32, kind="ExternalOutput")

    with tile.TileContext(nc) as tc:
        with tc.tile_pool(name="const", bufs=1) as cpool, \
             tc.tile_pool(name="pp", bufs=1) as pp, \
             tc.tile_pool(name="rf2", bufs=2) as rf2, \
             tc.tile_pool(name="rps1", bufs=2, space="PSUM") as rps1, \
             tc.tile_pool(name="rps2", bufs=2, space="PSUM") as rps2:
            ident = cpool.tile([P, P], f32, tag="ident")
            make_identity(nc, ident[:])
            iota = cpool.tile([P, P], f32, tag="iota")
            nc.sync.dma_start(iota[:], d_iota[:, :])
            woutr = cpool.tile([P, F], f32, tag="woutr")
            nc.sync.dma_start(woutr[:], d_woutr[:, :])
            w11 = cpool.tile([F, F], f32, tag="w11")
            nc.sync.dma_start(w11[:], d_w11[:, :])
            w21 = cpool.tile([F, F], f32, tag="w21")
            nc.sync.dma_start(w21[:], d_w21[:, :])
            dstloc = pp.tile([P, T], f32, tag="dstloc")
            nc.sync.dma_start(dstloc[:], d_dstloc[:, :])
            g_all = pp.tile([P, T, F], f32, tag="g_all")
            nc.sync.dma_start(g_all[:], d_gall[:, :].rearrange("p (t f) -> p t f", f=F))
            epat = pp.tile([P, NB], f32, tag="epat")
            nc.sync.dma_start(epat[:], d_epat[:, :])

                # ---------------- pass 2 ----------------
            with tc.tile_pool(name="p2", bufs=1) as p2, \
                 tc.tile_pool(name="rot2", bufs=3) as rot2, \
                 tc.tile_pool(name="p2ps", bufs=2, space="PSUM") as p2ps, \
                 tc.tile_pool(name="p2psm", bufs=1, space="PSUM") as p2psm:
                    x0src = p2.tile([P, T, F], f32, tag="x0src")
                    nc.sync.dma_start(x0src[:], d_x0src[:, :].rearrange(
                        "p (t c) -> p t c", c=F))
                    X1sb = p2.tile([P, NB, F], f32, tag="X1sb")
                    for b in range(NB):
                        x1ps = p2ps.tile([P, F], f32, tag="x1ps")
                        for j in range(T_blk):
                            t = b * T_blk + j
                            oh = rot2.tile([P, P], f32, tag="oh2")
                            nc.vector.tensor_scalar(
                                out=oh[:], in0=iota[:],
                                scalar1=dstloc[:, t:t + 1],
                                scalar2=None, op0=ALU.is_equal)
                            msg = rot2.tile([P, F], f32, tag="msg2")
                            nc.vector.tensor_tensor(out=msg[:],
                                                    in0=g_all[:, t, :],
                                                    in1=x0src[:, t, :],
                                                    op=ALU.mult)
                            nc.tensor.matmul(out=x1ps[:], lhsT=oh[:],
                                             rhs=msg[:], start=(j == 0),
                                             stop=(j == T_blk - 1))
                        nc.scalar.copy(out=X1sb[:, b, :], in_=x1ps[:])

                    # refinement 1 (gate = silu(h) only) + readout
                    segloc_t = p2.tile([P, NB], f32, tag="segloc")
                    nc.sync.dma_start(segloc_t[:], d_segloc[:, :])
                    amask_t = p2.tile([P, NB], f32, tag="amask")
                    nc.sync.dma_start(amask_t[:], d_amask[:, :])
                    bout_t = p2.tile([P, NB], f32, tag="bout")
                    nc.sync.dma_start(bout_t[:], d_boutg[:, :])
                    molps = p2psm.tile([P, 1], f32, tag="molps")
                    for b in range(NB):
                        trp = rps1.tile([F, P], f32, tag="trp")
                        nc.tensor.transpose(out=trp[:], in_=X1sb[:, b, :],
                                            identity=ident[:])
                        xT = rf2.tile([F, P], f32, tag="xT2")
                        nc.scalar.copy(out=xT[:], in_=trp[:])
                        hps = rps2.tile([P, F], f32, tag="hps")
                        nc.tensor.matmul(out=hps[:], lhsT=xT[:], rhs=w11[:],
                                         start=True, stop=True)
                        sw = rf2.tile([P, F], f32, tag="sw2")
                        nc.scalar.activation(out=sw[:], in_=hps[:],
                                             func=ACT.Silu)
                        gtp = rps1.tile([F, P], f32, tag="trp")
                        nc.tensor.transpose(out=gtp[:], in_=sw[:],
                                            identity=ident[:])
                        gT = rf2.tile([F, P], f32, tag="gT2")
                        nc.scalar.copy(out=gT[:], in_=gtp[:])
                        dps = rps2.tile([P, F], f32, tag="hps")
                        nc.tensor.matmul(out=dps[:], lhsT=gT[:], rhs=w21[:],
                                         start=True, stop=True)
                        x0b = rf2.tile([P, F], f32, tag="x0b")
                        nc.vector.tensor_tensor(out=x0b[:], in0=X1sb[:, b, :],
                                                in1=dps[:], op=ALU.add)
                        # e_atom
                        tmp2 = rf2.tile([P, F], f32, tag="tmp2")
                        nc.vector.tensor_tensor(out=tmp2[:], in0=x0b[:],
                                                in1=woutr[:], op=ALU.mult)
                        ea = rf2.tile([P, 1], f32, tag="ea")
                        nc.vector.tensor_reduce(out=ea[:], in_=tmp2[:],
                                                axis=mybir.AxisListType.X,
                                                op=ALU.add)
                        nc.vector.tensor_tensor(out=ea[:], in0=ea[:],
                                                in1=bout_t[:, b:b + 1],
                                                op=ALU.add)
                        nc.vector.tensor_tensor(out=ea[:], in0=ea[:],
                                                in1=epat[:, b:b + 1],
                                                op=ALU.add)
                        nc.vector.tensor_tensor(out=ea[:], in0=ea[:],
                                                in1=amask_t[:, b:b + 1],
                                                op=ALU.mult)
                        ohm = rf2.tile([P, P], f32, tag="ohm")
                        nc.vector.tensor_scalar(out=ohm[:], in0=iota[:],
                                                scalar1=segloc_t[:, b:b + 1],
                                                scalar2=None, op0=ALU.is_equal)
                        nc.tensor.matmul(out=molps[:], lhsT=ohm[:], rhs=ea[:],
                                         start=(b == 0), stop=(b == NB - 1))
                    mol = p2.tile([P, 1], f32, tag="mol")
                    nc.vector.tensor_copy(out=mol[:], in_=molps[:])
                    nc.sync.dma_start(d_out[:, :], mol[:])
    return nc


def kernel(**inputs):
    batch_mask = np.asarray(inputs["batch_mask"], np.float32)
    per_core, T, T_blk, mol_base, srci_arr = _host_prep(
        inputs["atomic_numbers"], inputs["positions"], inputs["dst_idx"],
        inputs["src_idx"], inputs["batch_segments"], inputs["batch_mask"],
        inputs["atom_mask"], inputs["embed"], inputs["Wr1_0"], inputs["Wr2_0"],
        inputs["W1_0"], inputs["W2_0"], inputs["Wr1_1"], inputs["W1_1"],
        inputs["W2_1"], inputs["w_out"], inputs["b_out"])

    key = (T, T_blk)
    if key not in _CACHE:
        ncA = _build_A(T, T_blk)
        ncA.finalize()
        ncB = _build_B(T, T_blk)
        ncB.finalize()
        _CACHE[key] = (ncA, ncB)
    ncA, ncB = _CACHE[key]

    from concourse.bass_utils import run_bass_kernel_spmd
    resA = run_bass_kernel_spmd(ncA, per_core, core_ids=list(range(NCORES)))

    x0full = np.zeros((N, F), dtype=np.float32)
    for c in range(NCORES):
        x0c = np.asarray(resA.results[c]["x0out"]).reshape(P, NB, F)
        x0full[c * AC:(c + 1) * AC] = x0c.transpose(1, 0, 2).reshape(AC, F)

    per_core_b = []
    for c in range(NCORES):
        pc = per_core[c]
        per_core_b.append({
            "dstloc": pc["dstloc"],
            "gall": np.asarray(resA.results[c]["gout"]),
            "epat_i": np.asarray(resA.results[c]["epat_o"]),
            "x0src": x0full[srci_arr[c]].reshape(P, -1),
            "iota_rep": pc["iota_rep"],
            "wout_rep": pc["wout_rep"], "w11": pc["w11"], "w21": pc["w21"],
            "boutg": pc["boutg"], "segloc": pc["segloc"], "amask": pc["amask"],
        })
    resB = run_bass_kernel_spmd(ncB, per_core_b, core_ids=list(range(NCORES)))
    out = np.zeros((B,), dtype=np.float32)
    for c in range(NCORES):
        w = np.asarray(resB.results[c]["out"]).reshape(-1)
        lo = int(mol_base[c])
        hi = min(lo + P, B)
        out[lo:hi] += w[:hi - lo]
    return out * batch_mask


def profile_exec_ns(**inputs):
    """Re-run both launches with NTFF tracing and return summed exec_time_ns."""
    per_core, T, T_blk, mol_base, srci_arr = _host_prep(
        inputs["atomic_numbers"], inputs["positions"], inputs["dst_idx"],
        inputs["src_idx"], inputs["batch_segments"], inputs["batch_mask"],
        inputs["atom_mask"], inputs["embed"], inputs["Wr1_0"], inputs["Wr2_0"],
        inputs["W1_0"], inputs["W2_0"], inputs["Wr1_1"], inputs["W1_1"],
        inputs["W2_1"], inputs["w_out"], inputs["b_out"])
    ncA, ncB = _CACHE[(T, T_blk)]
    from concourse.bass_utils import run_bass_kernel_spmd
    resA = run_bass_kernel_spmd(ncA, per_core, core_ids=list(range(NCORES)),
                                trace=True)
    if resA.exec_time_ns is None:
        raise RuntimeError("no exec_time_ns from trace (axon NTFF hook absent)")
    x0full = np.zeros((N, F), dtype=np.float32)
    for c in range(NCORES):
        x0c = np.asarray(resA.results[c]["x0out"]).reshape(P, NB, F)
        x0full[c * AC:(c + 1) * AC] = x0c.transpose(1, 0, 2).reshape(AC, F)
    per_core_b = []
    for c in range(NCORES):
        pc = per_core[c]
        per_core_b.append({
            "dstloc": pc["dstloc"],
            "gall": np.asarray(resA.results[c]["gout"]),
            "epat_i": np.asarray(resA.results[c]["epat_o"]),
            "x0src": x0full[srci_arr[c]].reshape(P, -1),
            "iota_rep": pc["iota_rep"],
            "wout_rep": pc["wout_rep"], "w11": pc["w11"], "w21": pc["w21"],
            "boutg": pc["boutg"], "segloc": pc["segloc"], "amask": pc["amask"],
        })
    resB = run_bass_kernel_spmd(ncB, per_core_b, core_ids=list(range(NCORES)),
                                trace=True)
    if resB.exec_time_ns is None:
        raise RuntimeError("no exec_time_ns from trace for pass B")
    return int(resA.exec_time_ns) + int(resB.exec_time_ns)



# revision 7
# speedup vs baseline: 15.9210x; 15.9210x over previous
"""Bass/Trainium2 kernel for nn_EF_42511586295882 (GNN message passing).

Math reduction proven against reference: only the l=0 spherical channel of
iteration 0 reaches the output.  See kernel docstring history; computation:

  rad[e,k]  = T_k(2*exp(-r)-1) * cut(r)                        (E,16)
  msg0[e,f] = (rad @ (0.282095*Wr1_0 + Wr2_0))[e,f] * embed[z[src_e], f]
  X0[a,f]   = sum_{e: dst=a} msg0[e,f]
  x0        = X0 + (h0 * silu(h0)) @ W2_0,   h0 = X0 @ W1_0
  msg1[e,f] = (rad @ Wr1_1)[e,f] * x0[src_e, f]
  X1[a,f]   = sum_{e: dst=a} msg1[e,f]
  x0b       = X1 + silu(X1 @ W1_1) @ W2_1
  e_atom    = x0b @ w_out + b_out[z] + sum_{e: dst=a} e_pair[e]
  e_mol     = segment_sum(e_atom * atom_mask, batch_segments)

Single-launch design (the measured quantity is warm wall-clock; data
transfer and per-call jit overhead dominate, device compute is ~ms):
ship 2 packed tensors per core (f32 params blob + one i32/edge packed
index word srci<<16|dsti), unpack + gather everything on device with DVE
integer ops and indirect DMA, AllGather the sharded atom table and the
inter-pass x0 exchange on device, download only the (P,1) molecule sums.
The JAX persistent compilation cache is enabled so warm calls skip the
neuronx-cc recompile that run_bass_kernel_spmd otherwise pays per call
(fresh jax.jit per invocation).

Sharding: edges sorted by dst block; core k owns atoms [2048k, 2048(k+1))
and all edges into them, in 16 aligned 128-atom blocks.  Scatter = one-hot
matmul into a per-block PSUM accumulator.  Padded edge slots point at
sentinel table rows placed at +-1e6 so cut(r) underflows to exactly 0.
"""

import math
import os
import tempfile

import numpy as np

P = 128
N = 16384
E = 262144
B = 512
F = 32
K = 16
NZ = 119
NCORES = 8
AC = N // NCORES          # atoms per core
NB = AC // P              # 128-atom blocks per core (16)
AP_ROWS = AC + 128        # padded shard rows (multiple of 128; sentinels at AC, AC+1)
PAT_COLS = AP_ROWS * 8 // P   # 136
CUTOFF = 6.0
KE = 14.399645
ZBL_C = [0.18175, 0.50986, 0.28022, 0.02817]
ZBL_D = [3.19980, 0.94229, 0.40290, 0.20162]
A_PRE = 0.8854 * 0.529177

# f32 blob column offsets
C_WCAT = 0            # [P, 2F]
C_WOUT = 64           # [P, F]
C_W4 = 96             # 4x [F, F] packed in row groups
C_BOUT = 128          # [P, NB]
C_SEG = 144           # [P, NB]
C_AMASK = 160         # [P, NB]
C_PAT = 176           # [P, PAT_COLS] == patsh [AP_ROWS, 8] flat
C_EMB = 176 + PAT_COLS            # [P, F] == embed table rows
WF = C_EMB + F                    # 337 f32 cols

_CACHE = {}
_CFG_DONE = False


def _enable_jax_cache():
    global _CFG_DONE
    if _CFG_DONE:
        return
    _CFG_DONE = True
    try:
        import jax
        d = os.path.join(tempfile.gettempdir(), "jax_bass_cache_nn_ef")
        os.makedirs(d, exist_ok=True)
        jax.config.update("jax_compilation_cache_dir", d)
        jax.config.update("jax_persistent_cache_min_entry_size_bytes", -1)
        jax.config.update("jax_persistent_cache_min_compile_time_secs", 0.0)
    except Exception:
        pass  # cache is an optimization only; run without it on failure


def _host_prep(atomic_numbers, positions, dst_idx, src_idx, batch_segments,
               atom_mask, embed, Wr1_0, Wr2_0, W1_0, W2_0,
               Wr1_1, W1_1, W2_1, w_out, b_out):
    an = np.asarray(atomic_numbers).astype(np.int32)
    pos = np.asarray(positions, dtype=np.float32)
    dst = np.asarray(dst_idx).astype(np.int32)
    src = np.asarray(src_idx).astype(np.int32)
    seg = np.asarray(batch_segments).astype(np.int32)

    bucket0 = (dst >> 7).astype(np.uint8)         # 128-atom block id, 0..127
    order = np.argsort(bucket0, kind="stable")
    ds, ss = dst[order], src[order]

    bucket = bucket0[order].astype(np.int64)
    counts = np.bincount(bucket, minlength=NCORES * NB)
    T_blk = int(math.ceil(counts.max() / P))
    T = NB * T_blk

    start = np.zeros(NCORES * NB + 1, dtype=np.int64)
    np.cumsum(counts, out=start[1:])
    k = np.arange(E, dtype=np.int64) - start[bucket]
    p = k % P
    tcol = (bucket % NB) * T_blk + k // P
    c = bucket >> 4

    # device-table row for atom n: (n // AC) * AP_ROWS + n % AC  (< 17408)
    # packed: srci_dev << 16 | dsti_dev; dstloc = dsti_dev & 127 (AP_ROWS % 128 == 0)
    idx32 = np.full((NCORES, P, T), ((AC + 1) << 16) | AC, dtype=np.int32)
    dsti_dev = (ds >> 11) * AP_ROWS + (ds & (AC - 1))
    srci_dev = (ss >> 11) * AP_ROWS + (ss & (AC - 1))
    idx32[c, p, tcol] = ((srci_dev << 16) | dsti_dev).astype(np.int32)

    blob = np.zeros((NCORES, P, WF), dtype=np.float32)

    gcW = 0.282095 * np.asarray(Wr1_0, np.float32) + np.asarray(Wr2_0, np.float32)
    for j in range(4):
        blob[:, 32 * j:32 * j + K, C_WCAT:C_WCAT + F] = gcW
        blob[:, 32 * j:32 * j + K, C_WCAT + F:C_WCAT + 2 * F] = np.asarray(Wr1_1, np.float32)
    blob[:, :, C_WOUT:C_WOUT + F] = np.asarray(w_out, np.float32)[None, None, :]
    for j, w in enumerate([W1_0, W2_0, W1_1, W2_1]):
        blob[:, 32 * j:32 * j + F, C_W4:C_W4 + F] = np.asarray(w, np.float32)

    ownz = an.reshape(NCORES, NB, P).transpose(0, 2, 1)
    segv = seg.reshape(NCORES, NB, P).transpose(0, 2, 1)
    mol_base = segv.min(axis=(1, 2))
    segloc = (segv - mol_base[:, None, None]).astype(np.float32)
    assert segloc.max() < P, "molecule window exceeds 128 per core"
    blob[:, :, C_BOUT:C_BOUT + NB] = np.asarray(b_out, np.float32)[ownz]
    blob[:, :, C_SEG:C_SEG + NB] = segloc
    blob[:, :, C_AMASK:C_AMASK + NB] = (
        np.asarray(atom_mask, np.float32).reshape(NCORES, NB, P).transpose(0, 2, 1))

    # atom table shard [AP_ROWS, 8]: [px,py,pz,zf,zpow,0,0,0]; sentinels
    zpow_tab = (np.arange(NZ, dtype=np.float32) ** 0.23).astype(np.float32)
    patsh = np.zeros((NCORES, AP_ROWS, 8), dtype=np.float32)
    patsh[:, :AC, 0:3] = pos.reshape(NCORES, AC, 3)
    anc = an.reshape(NCORES, AC)
    patsh[:, :AC, 3] = anc
    patsh[:, :AC, 4] = zpow_tab[anc]
    patsh[:, AC, 0] = 1e6
    patsh[:, AC + 1, 0] = -1e6
    blob[:, :, C_PAT:C_PAT + PAT_COLS] = patsh.reshape(NCORES, P, PAT_COLS)

    blob[:, :NZ, C_EMB:C_EMB + F] = np.asarray(embed, dtype=np.float32)[None, :, :]

    per_core = [{"blob": blob[cc], "idx32": idx32[cc]} for cc in range(NCORES)]
    return per_core, T, T_blk, mol_base


def _build(T, T_blk):
    import concourse.bacc as bacc
    import concourse.bass as bass
    import concourse.mybir as mybir
    import concourse.tile as tile
    from concourse.masks import make_identity

    f32 = mybir.dt.float32
    i32 = mybir.dt.int32
    ALU = mybir.AluOpType
    ACT = mybir.ActivationFunctionType

    nc = bacc.Bacc("TRN2", target_bir_lowering=False, debug=False,
                   num_devices=NCORES)

    d_blob = nc.dram_tensor("blob", [P, WF], f32, kind="ExternalInput")
    d_idx32 = nc.dram_tensor("idx32", [P, T], i32, kind="ExternalInput")
    d_out = nc.dram_tensor("out", [P, 1], f32, kind="ExternalOutput")

    with tile.TileContext(nc) as tc:
        with tc.tile_pool(name="const", bufs=1) as cpool, \
             tc.tile_pool(name="persist", bufs=1) as pp, \
             tc.tile_pool(name="dram", bufs=1, space="DRAM") as dpool:

            # ---- device-side atom table: shard -> bounce -> AllGather ----
            patloc = dpool.tile([AP_ROWS, 8], f32, tag="patloc")
            nc.sync.dma_start(patloc[:], d_blob[:, C_PAT:C_PAT + PAT_COLS])
            patall = dpool.tile([NCORES * AP_ROWS, 8], f32, tag="patall",
                                addr_space="Shared")
            nc.gpsimd.collective_compute(
                "AllGather", mybir.AluOpType.bypass,
                replica_groups=[list(range(NCORES))],
                ins=[patloc[:]], outs=[patall[:]])
            embedt = dpool.tile([P, F], f32, tag="embedt")
            nc.sync.dma_start(embedt[:], d_blob[:, C_EMB:C_EMB + F])

            ident = cpool.tile([P, P], f32, tag="ident")
            make_identity(nc, ident[:])
            iotai = cpool.tile([P, P], i32, tag="iotai")
            nc.gpsimd.iota(iotai[:], pattern=[[1, P]], base=0,
                           channel_multiplier=0)
            iota = cpool.tile([P, P], f32, tag="iota")
            nc.vector.tensor_copy(out=iota[:], in_=iotai[:])
            wcat = cpool.tile([P, 2 * F], f32, tag="wcat")
            nc.sync.dma_start(wcat[:], d_blob[:, C_WCAT:C_WCAT + 2 * F])
            woutr = cpool.tile([P, F], f32, tag="woutr")
            nc.sync.dma_start(woutr[:], d_blob[:, C_WOUT:C_WOUT + F])
            w10 = cpool.tile([F, F], f32, tag="w10")
            nc.sync.dma_start(w10[:], d_blob[0:F, C_W4:C_W4 + F])
            w20 = cpool.tile([F, F], f32, tag="w20")
            nc.sync.dma_start(w20[:], d_blob[F:2 * F, C_W4:C_W4 + F])
            w11 = cpool.tile([F, F], f32, tag="w11")
            nc.sync.dma_start(w11[:], d_blob[2 * F:3 * F, C_W4:C_W4 + F])
            w21 = cpool.tile([F, F], f32, tag="w21")
            nc.sync.dma_start(w21[:], d_blob[3 * F:4 * F, C_W4:C_W4 + F])

            idxp = pp.tile([P, T], i32, tag="idxp")
            nc.sync.dma_start(idxp[:], d_idx32[:, :])
            srci_t = pp.tile([P, T], i32, tag="srci_t")
            nc.vector.tensor_scalar(out=srci_t[:], in0=idxp[:], scalar1=16,
                                    scalar2=None, op0=ALU.logical_shift_right)
            dloci = pp.tile([P, T], i32, tag="dloci")
            nc.vector.tensor_scalar(out=dloci[:], in0=idxp[:], scalar1=P - 1,
                                    scalar2=None, op0=ALU.bitwise_and)
            dstloc = pp.tile([P, T], f32, tag="dstloc")
            nc.vector.tensor_copy(out=dstloc[:], in_=dloci[:])

            g_all = pp.tile([P, T, F], f32, tag="g_all")
            epair = pp.tile([P, T], f32, tag="epair")
            X0sb = pp.tile([P, NB, F], f32, tag="X0sb")
            epat = pp.tile([P, NB], f32, tag="epat")
            x0sb = pp.tile([P, NB, F], f32, tag="x0sb")

            # ---------------- pass 1: edge batch math ----------------
            with tc.tile_pool(name="p1", bufs=1) as p1, \
                 tc.tile_pool(name="rot", bufs=3) as rot, \
                 tc.tile_pool(name="ps1", bufs=2, space="PSUM") as ps_rt, \
                 tc.tile_pool(name="ps2", bufs=2, space="PSUM") as ps_g, \
                 tc.tile_pool(name="ps3", bufs=2, space="PSUM") as ps_x:

                dsti_t = p1.tile([P, T], i32, tag="dsti_t")
                nc.vector.tensor_scalar(out=dsti_t[:], in0=idxp[:],
                                        scalar1=0xFFFF, scalar2=None,
                                        op0=ALU.bitwise_and)

                pd = p1.tile([P, T, 8], f32, tag="pd")
                ps_ = p1.tile([P, T, 8], f32, tag="ps")
                for t in range(T):
                    nc.gpsimd.indirect_dma_start(
                        out=pd[:, t, :], out_offset=None, in_=patall[:],
                        in_offset=bass.IndirectOffsetOnAxis(
                            ap=dsti_t[:, t:t + 1], axis=0))
                    nc.gpsimd.indirect_dma_start(
                        out=ps_[:, t, :], out_offset=None, in_=patall[:],
                        in_offset=bass.IndirectOffsetOnAxis(
                            ap=srci_t[:, t:t + 1], axis=0))

                # embed gather by src atomic number (exact small ints in f32)
                izsrc = p1.tile([P, T], i32, tag="izsrc")
                nc.vector.tensor_copy(out=izsrc[:], in_=ps_[:, :, 3])
                xs0 = p1.tile([P, T, F], f32, tag="xs0")
                for t in range(T):
                    nc.gpsimd.indirect_dma_start(
                        out=xs0[:, t, :], out_offset=None, in_=embedt[:],
                        in_offset=bass.IndirectOffsetOnAxis(
                            ap=izsrc[:, t:t + 1], axis=0))

                disp = p1.tile([P, T, 3], f32, tag="disp")
                nc.vector.tensor_tensor(out=disp[:], in0=ps_[:, :, 0:3],
                                        in1=pd[:, :, 0:3], op=ALU.subtract)
                sq = p1.tile([P, T, 3], f32, tag="sq")
                nc.vector.tensor_tensor(out=sq[:], in0=disp[:], in1=disp[:],
                                        op=ALU.mult)
                r2 = p1.tile([P, T], f32, tag="r2")
                nc.vector.tensor_reduce(out=r2[:], in_=sq[:],
                                        axis=mybir.AxisListType.X, op=ALU.add)
                r = p1.tile([P, T], f32, tag="r")
                nc.scalar.activation(out=r[:], in_=r2[:], func=ACT.Sqrt)
                nc.vector.tensor_scalar_max(out=r[:], in0=r[:], scalar1=1e-4)

                # t = 2*exp(-r) - 1
                tch = p1.tile([P, T], f32, tag="tch")
                nc.scalar.activation(out=tch[:], in_=r[:], func=ACT.Exp,
                                     scale=-1.0)
                t2 = p1.tile([P, T], f32, tag="t2")
                nc.vector.tensor_scalar(out=t2[:], in0=tch[:], scalar1=4.0,
                                        scalar2=-2.0, op0=ALU.mult, op1=ALU.add)
                nc.vector.tensor_scalar(out=tch[:], in0=tch[:], scalar1=2.0,
                                        scalar2=-1.0, op0=ALU.mult, op1=ALU.add)

                # cutoff: cut = exp(-u2/(1-u2)), u = min(r/C, 1-1e-6)
                u = p1.tile([P, T], f32, tag="u")
                nc.vector.tensor_scalar(out=u[:], in0=r[:],
                                        scalar1=1.0 / CUTOFF,
                                        scalar2=1.0 - 1e-6,
                                        op0=ALU.mult, op1=ALU.min)
                u2 = p1.tile([P, T], f32, tag="u2")
                nc.vector.tensor_tensor(out=u2[:], in0=u[:], in1=u[:],
                                        op=ALU.mult)
                den = p1.tile([P, T], f32, tag="den")
                nc.vector.tensor_scalar(out=den[:], in0=u2[:], scalar1=-1.0,
                                        scalar2=1.0, op0=ALU.mult, op1=ALU.add)
                nc.vector.reciprocal(out=den[:], in_=den[:])
                frac = p1.tile([P, T], f32, tag="frac")
                nc.vector.tensor_tensor(out=frac[:], in0=u2[:], in1=den[:],
                                        op=ALU.mult)
                cutm = p1.tile([P, T], f32, tag="cutm")
                nc.scalar.activation(out=cutm[:], in_=frac[:], func=ACT.Exp,
                                     scale=-1.0)

                # Chebyshev ladder, seeded with cutm so rad_k = T_k(t)*cut
                rad = p1.tile([P, T, 2 * K], f32, tag="rad")
                nc.vector.memset(rad[:], 0.0)
                nc.vector.tensor_copy(out=rad[:, :, 0], in_=cutm[:])
                nc.vector.tensor_tensor(out=rad[:, :, 1], in0=tch[:],
                                        in1=cutm[:], op=ALU.mult)
                tmp = p1.tile([P, T], f32, tag="tmp")
                for k in range(2, K):
                    nc.vector.tensor_tensor(out=tmp[:], in0=t2[:],
                                            in1=rad[:, :, k - 1], op=ALU.mult)
                    nc.vector.tensor_tensor(out=rad[:, :, k], in0=tmp[:],
                                            in1=rad[:, :, k - 2],
                                            op=ALU.subtract)

                # ---- ZBL pair energy ----
                zz = p1.tile([P, T], f32, tag="zz")
                nc.vector.tensor_tensor(out=zz[:], in0=pd[:, :, 3],
                                        in1=ps_[:, :, 3], op=ALU.mult)
                asum = p1.tile([P, T], f32, tag="asum")
                nc.vector.tensor_tensor(out=asum[:], in0=pd[:, :, 4],
                                        in1=ps_[:, :, 4], op=ALU.add)
                nc.vector.tensor_scalar_add(out=asum[:], in0=asum[:],
                                            scalar1=1e-10)
                ra = p1.tile([P, T], f32, tag="ra")
                nc.vector.tensor_tensor(out=ra[:], in0=r[:], in1=asum[:],
                                        op=ALU.mult)
                nc.vector.tensor_scalar_mul(out=ra[:], in0=ra[:],
                                            scalar1=1.0 / A_PRE)
                phi = p1.tile([P, T], f32, tag="phi")
                ej = p1.tile([P, T], f32, tag="ej")
                for j in range(4):
                    nc.scalar.activation(out=ej[:], in_=ra[:], func=ACT.Exp,
                                         scale=-ZBL_D[j])
                    if j == 0:
                        nc.vector.tensor_scalar_mul(out=phi[:], in0=ej[:],
                                                    scalar1=ZBL_C[j])
                    else:
                        nc.vector.tensor_scalar(out=ej[:], in0=ej[:],
                                                scalar1=ZBL_C[j], scalar2=None,
                                                op0=ALU.mult)
                        nc.vector.tensor_tensor(out=phi[:], in0=phi[:],
                                                in1=ej[:], op=ALU.add)
                rinv = p1.tile([P, T], f32, tag="rinv")
                nc.vector.reciprocal(out=rinv[:], in_=r[:])
                nc.vector.tensor_tensor(out=epair[:], in0=zz[:], in1=phi[:],
                                        op=ALU.mult)
                nc.vector.tensor_tensor(out=epair[:], in0=epair[:], in1=rinv[:],
                                        op=ALU.mult)
                nc.vector.tensor_tensor(out=epair[:], in0=epair[:], in1=cutm[:],
                                        op=ALU.mult)
                nc.vector.tensor_scalar_mul(out=epair[:], in0=epair[:],
                                            scalar1=0.5 * KE)

                # ---------------- pass 1: per-tile scatter ----------------
                for b in range(NB):
                    x0ps = ps_x.tile([P, F + 1], f32, tag="x0ps")
                    for j in range(T_blk):
                        t = b * T_blk + j
                        g4 = t % 4
                        if g4 == 0:
                            radT = ps_rt.tile([P, P], f32, tag="radT")
                            hi = min(4, T - t)
                            nc.tensor.transpose(
                                out=radT[0:32 * hi, :],
                                in_=rad[:, t:t + hi, :],
                                identity=ident[:])
                            radTs = rot.tile([P, P], f32, tag="radTs")
                            nc.scalar.copy(out=radTs[0:32 * hi, :],
                                           in_=radT[0:32 * hi, :])
                        gps = ps_g.tile([P, 2 * F], f32, tag="gps")
                        nc.tensor.matmul(out=gps[:],
                                         lhsT=radTs[32 * g4:32 * g4 + 32, :],
                                         rhs=wcat[32 * g4:32 * g4 + 32, :],
                                         start=True, stop=True,
                                         tile_position=(32 * g4, 0))
                        oh = rot.tile([P, P], f32, tag="oh")
                        nc.vector.tensor_scalar(out=oh[:], in0=iota[:],
                                                scalar1=dstloc[:, t:t + 1],
                                                scalar2=None, op0=ALU.is_equal)
                        msg = rot.tile([P, F + 1], f32, tag="msg")
                        nc.vector.tensor_tensor(out=msg[:, 0:F], in0=gps[:, 0:F],
                                                in1=xs0[:, t, :], op=ALU.mult)
                        nc.vector.tensor_copy(out=msg[:, F:F + 1],
                                              in_=epair[:, t:t + 1])
                        nc.scalar.copy(out=g_all[:, t, :], in_=gps[:, F:2 * F])
                        nc.tensor.matmul(out=x0ps[:], lhsT=oh[:], rhs=msg[:],
                                         start=(j == 0), stop=(j == T_blk - 1))
                    nc.scalar.copy(out=X0sb[:, b, :], in_=x0ps[:, 0:F])
                    nc.vector.tensor_copy(out=epat[:, b:b + 1],
                                          in_=x0ps[:, F:F + 1])

            # ---------------- refinement 0 ----------------
            with tc.tile_pool(name="rf", bufs=2) as rf, \
                 tc.tile_pool(name="rps1", bufs=2, space="PSUM") as rps1, \
                 tc.tile_pool(name="rps2", bufs=2, space="PSUM") as rps2:
                for b in range(NB):
                    trp = rps1.tile([F, P], f32, tag="trp")
                    nc.tensor.transpose(out=trp[:], in_=X0sb[:, b, :],
                                        identity=ident[:])
                    xT = rf.tile([F, P], f32, tag="xT")
                    nc.scalar.copy(out=xT[:], in_=trp[:])
                    hps = rps2.tile([P, F], f32, tag="hps")
                    nc.tensor.matmul(out=hps[:], lhsT=xT[:], rhs=w10[:],
                                     start=True, stop=True)
                    sw = rf.tile([P, F], f32, tag="sw")
                    nc.scalar.activation(out=sw[:], in_=hps[:], func=ACT.Silu)
                    gate = rf.tile([P, F], f32, tag="gate")
                    nc.vector.tensor_tensor(out=gate[:], in0=hps[:], in1=sw[:],
                                            op=ALU.mult)
                    gtp = rps1.tile([F, P], f32, tag="trp")
                    nc.tensor.transpose(out=gtp[:], in_=gate[:],
                                        identity=ident[:])
                    gT = rf.tile([F, P], f32, tag="gT")
                    nc.scalar.copy(out=gT[:], in_=gtp[:])
                    dps = rps2.tile([P, F], f32, tag="hps")
                    nc.tensor.matmul(out=dps[:], lhsT=gT[:], rhs=w20[:],
                                     start=True, stop=True)
                    nc.vector.tensor_tensor(out=x0sb[:, b, :],
                                            in0=X0sb[:, b, :], in1=dps[:],
                                            op=ALU.add)

            # ---------------- x0 exchange: AllGather ----------------
            x0loc = dpool.tile([AP_ROWS, F], f32, tag="x0loc")
            for b in range(NB):
                nc.sync.dma_start(x0loc[b * P:(b + 1) * P, :], x0sb[:, b, :])
            with tc.tile_pool(name="zp", bufs=1) as zp:
                z128 = zp.tile([P, F], f32, tag="z128")
                nc.vector.memset(z128[:], 0.0)
                nc.sync.dma_start(x0loc[AC:AC + P, :], z128[:])
            x0all = dpool.tile([NCORES * AP_ROWS, F], f32, tag="x0all",
                               addr_space="Shared")
            nc.gpsimd.collective_compute(
                "AllGather", mybir.AluOpType.bypass,
                replica_groups=[list(range(NCORES))],
                ins=[x0loc[:]], outs=[x0all[:]])

            # ---------------- pass 2 ----------------
            with tc.tile_pool(name="p2", bufs=1) as p2, \
                 tc.tile_pool(name="rot2", bufs=3) as rot2, \
                 tc.tile_pool(name="p2ps", bufs=2, space="PSUM") as p2ps, \
                 tc.tile_pool(name="rps1", bufs=2, space="PSUM") as rps1, \
                 tc.tile_pool(name="rps2", bufs=2, space="PSUM") as rps2, \
                 tc.tile_pool(name="rf2", bufs=2) as rf2, \
                 tc.tile_pool(name="p2psm", bufs=1, space="PSUM") as p2psm:
                x0src = p2.tile([P, T, F], f32, tag="x0src")
                for t in range(T):
                    nc.gpsimd.indirect_dma_start(
                        out=x0src[:, t, :], out_offset=None, in_=x0all[:],
                        in_offset=bass.IndirectOffsetOnAxis(
                            ap=srci_t[:, t:t + 1], axis=0))
                X1sb = p2.tile([P, NB, F], f32, tag="X1sb")
                for b in range(NB):
                    x1ps = p2ps.tile([P, F], f32, tag="x1ps")
                    for j in range(T_blk):
                        t = b * T_blk + j
                        oh = rot2.tile([P, P], f32, tag="oh2")
                        nc.vector.tensor_scalar(
                            out=oh[:], in0=iota[:],
                            scalar1=dstloc[:, t:t + 1],
                            scalar2=None, op0=ALU.is_equal)
                        msg = rot2.tile([P, F], f32, tag="msg2")
                        nc.vector.tensor_tensor(out=msg[:],
                                                in0=g_all[:, t, :],
                                                in1=x0src[:, t, :],
                                                op=ALU.mult)
                        nc.tensor.matmul(out=x1ps[:], lhsT=oh[:],
                                         rhs=msg[:], start=(j == 0),
                                         stop=(j == T_blk - 1))
                    nc.scalar.copy(out=X1sb[:, b, :], in_=x1ps[:])

                # refinement 1 (gate = silu(h) only) + readout
                segloc_t = p2.tile([P, NB], f32, tag="segloc_t")
                nc.sync.dma_start(segloc_t[:], d_blob[:, C_SEG:C_SEG + NB])
                amask_t = p2.tile([P, NB], f32, tag="amask_t")
                nc.sync.dma_start(amask_t[:], d_blob[:, C_AMASK:C_AMASK + NB])
                bout_t = p2.tile([P, NB], f32, tag="bout_t")
                nc.sync.dma_start(bout_t[:], d_blob[:, C_BOUT:C_BOUT + NB])
                molps = p2psm.tile([P, 1], f32, tag="molps")
                for b in range(NB):
                    trp = rps1.tile([F, P], f32, tag="trp")
                    nc.tensor.transpose(out=trp[:], in_=X1sb[:, b, :],
                                        identity=ident[:])
                    xT = rf2.tile([F, P], f32, tag="xT2")
                    nc.scalar.copy(out=xT[:], in_=trp[:])
                    hps = rps2.tile([P, F], f32, tag="hps")
                    nc.tensor.matmul(out=hps[:], lhsT=xT[:], rhs=w11[:],
                                     start=True, stop=True)
                    sw = rf2.tile([P, F], f32, tag="sw2")
                    nc.scalar.activation(out=sw[:], in_=hps[:],
                                         func=ACT.Silu)
                    gtp = rps1.tile([F, P], f32, tag="trp")
                    nc.tensor.transpose(out=gtp[:], in_=sw[:],
                                        identity=ident[:])
                    gT = rf2.tile([F, P], f32, tag="gT2")
                    nc.scalar.copy(out=gT[:], in_=gtp[:])
                    dps = rps2.tile([P, F], f32, tag="hps")
                    nc.tensor.matmul(out=dps[:], lhsT=gT[:], rhs=w21[:],
                                     start=True, stop=True)
                    x0b = rf2.tile([P, F], f32, tag="x0b")
                    nc.vector.tensor_tensor(out=x0b[:], in0=X1sb[:, b, :],
                                            in1=dps[:], op=ALU.add)
                    # e_atom
                    tmp2 = rf2.tile([P, F], f32, tag="tmp2")
                    nc.vector.tensor_tensor(out=tmp2[:], in0=x0b[:],
                                            in1=woutr[:], op=ALU.mult)
                    ea = rf2.tile([P, 1], f32, tag="ea")
                    nc.vector.tensor_reduce(out=ea[:], in_=tmp2[:],
                                            axis=mybir.AxisListType.X,
                                            op=ALU.add)
                    nc.vector.tensor_tensor(out=ea[:], in0=ea[:],
                                            in1=bout_t[:, b:b + 1],
                                            op=ALU.add)
                    nc.vector.tensor_tensor(out=ea[:], in0=ea[:],
                                            in1=epat[:, b:b + 1],
                                            op=ALU.add)
                    nc.vector.tensor_tensor(out=ea[:], in0=ea[:],
                                            in1=amask_t[:, b:b + 1],
                                            op=ALU.mult)
                    ohm = rf2.tile([P, P], f32, tag="ohm")
                    nc.vector.tensor_scalar(out=ohm[:], in0=iota[:],
                                            scalar1=segloc_t[:, b:b + 1],
                                            scalar2=None, op0=ALU.is_equal)
                    nc.tensor.matmul(out=molps[:], lhsT=ohm[:], rhs=ea[:],
                                     start=(b == 0), stop=(b == NB - 1))
                mol = p2.tile([P, 1], f32, tag="mol")
                nc.vector.tensor_copy(out=mol[:], in_=molps[:])
                nc.sync.dma_start(d_out[:, :], mol[:])
    return nc


def kernel(**inputs):
    _enable_jax_cache()
    batch_mask = np.asarray(inputs["batch_mask"], np.float32)
    per_core, T, T_blk, mol_base = _host_prep(
        inputs["atomic_numbers"], inputs["positions"], inputs["dst_idx"],
        inputs["src_idx"], inputs["batch_segments"],
        inputs["atom_mask"], inputs["embed"], inputs["Wr1_0"], inputs["Wr2_0"],
        inputs["W1_0"], inputs["W2_0"], inputs["Wr1_1"], inputs["W1_1"],
        inputs["W2_1"], inputs["w_out"], inputs["b_out"])

    key = (T, T_blk)
    if key not in _CACHE:
        ncc = _build(T, T_blk)
        ncc.finalize()
        _CACHE[key] = ncc
    ncc = _CACHE[key]

    from concourse.bass_utils import run_bass_kernel_spmd
    res = run_bass_kernel_spmd(ncc, per_core, core_ids=list(range(NCORES)))

    out = np.zeros((B,), dtype=np.float32)
    for c in range(NCORES):
        w = np.asarray(res.results[c]["out"]).reshape(-1)
        lo = int(mol_base[c])
        hi = min(lo + P, B)
        out[lo:hi] += w[:hi - lo]
    return out * batch_mask


def profile_exec_ns(**inputs):
    """Re-run with NTFF tracing and return exec_time_ns (axon hook needed)."""
    _enable_jax_cache()
    per_core, T, T_blk, mol_base = _host_prep(
        inputs["atomic_numbers"], inputs["positions"], inputs["dst_idx"],
        inputs["src_idx"], inputs["batch_segments"],
        inputs["atom_mask"], inputs["embed"], inputs["Wr1_0"], inputs["Wr2_0"],
        inputs["W1_0"], inputs["W2_0"], inputs["Wr1_1"], inputs["W1_1"],
        inputs["W2_1"], inputs["w_out"], inputs["b_out"])
    ncc = _CACHE[(T, T_blk)]
    from concourse.bass_utils import run_bass_kernel_spmd
    res = run_bass_kernel_spmd(ncc, per_core, core_ids=list(range(NCORES)),
                               trace=True)
    if res.exec_time_ns is None:
        raise RuntimeError("no exec_time_ns from trace (axon NTFF hook absent)")
    return int(res.exec_time_ns)
